# revision 8
# baseline (speedup 1.0000x reference)
"""Multi-head attention (B=2, L=2048, DIM=1024, H=16) on 8 TRN2 NeuronCores.

Sharding: core c = (batch b = c//4, head-group hg = c%4 of 4 heads / 256 dims).
Data parallel over B, tensor parallel over heads; Q/K/V weights column-sharded.
Each core is fully independent (no collectives); host gathers the 8 output
shards.

Per-core layout trick: everything is computed transposed (seq on the free
axis) so no on-device transposes are needed:
  QT/KT [hd, seq]  <- matmul(lhsT=W_slice, rhs=xT)       (xT transposed on host)
  ST    [k, q]     <- matmul(lhsT=KT_head, rhs=QT_head)  (= scores transposed)
  E     = exp(ST)         (max-subtraction skipped: logits are N(0,1)-scaled,
                           mask only subtracts -> exp stays in [e^-65, e^5])
  Emask = E * exp(-60*mask)^T                            (mask exp'd on host)
  OT    [hd+ones, q] <- matmul(lhsT=[V | ones], rhs=Emask) accumulated over k;
                        rows 64..127 give the softmax denominator replicated,
                        so out = OT[0:64] / OT[64:128] needs no partition
                        broadcast.
The 1/sqrt(64) score scale is folded into Wq on the host.
Biases are zeros per the problem spec and are skipped.

Scheduling (v2): the PE is the bottleneck engine (~178us of matmul work),
so the kernel keeps it busy end to end:
  - Phase A streams the input DMA: per contraction block kd, the projection
    matmuls for KT(kp0), QT(j0) and V(kb0-3) are emitted right behind that
    block's loads, accumulating in 8 concurrently-live PSUM banks.  The PE
    starts ~2us after launch instead of waiting ~13us for the full 5.5MB
    of xT+W.  NOTE: PSUM accumulation groups are per-BANK — interleaving
    two open groups inside one bank corrupts results (measured), so every
    concurrently-accumulating group gets its own bank ([128,1024] tiles
    hold two groups only because their halves are bank-aligned).
  - The remaining projections (KT kp1-3, V kb4-15, QT j1-3) are "deferred
    units" emitted into the attention instruction stream on an explicit
    schedule (each unit lands before its first consumer), so panel
    boundaries never stall on a projection and the PE has no idle gaps.
  - The per-head normalization is batched per head-pair ([64,1024] Ln/Exp
    on ScalarE instead of 2x[64,512]) and the exp(-ln) trick keeps
    division off the DVE.
"""

import sys

for _p in ("/opt/trn_rl_repo",):
    if _p not in sys.path:
        sys.path.append(_p)

import numpy as np
import ml_dtypes

import concourse.tile as tile
from concourse import bacc, mybir
from concourse.bass_utils import run_bass_kernel_spmd


def _patch_act_tables():
    """Force every activation onto the one table set that holds both Exp
    and Ln, so the kernel pays a single ACT_TABLE_LOAD instead of
    thrashing between `exp_and_others` and `natural_log` at every
    normalization (measured 19 loads = ~24us).  Set ids must stay stable
    (they index act_info.json), so entries are kept and only their
    function sets are emptied.
    """
    import concourse.hw_specs as hw_specs

    orig = hw_specs.get_activation_tables

    def patched(arch):
        t = orig(arch)
        keep = "natural_log_exp_and_others"
        if keep not in t:
            return t
        return {k: (v if k == keep else set()) for k, v in t.items()}

    patched.__wrapped__ = orig
    bacc.get_activation_tables = patched


_patch_act_tables()

BF16 = ml_dtypes.bfloat16

B, L, DIM, H = 2, 2048, 1024, 16
HPC = 4          # heads per core
HD = DIM // H    # 64
GW = HPC * HD    # 256, head-group width per core
N_CORES = 8
MASK_SCALE = -60.0
SCALE = float(HD) ** -0.5

P = 128
KD = DIM // P        # 8  contraction blocks for projections
NSEQ = L // P        # 16 seq blocks (k blocks)
QP = 512             # q panel width
NQP = L // QP        # 4 q panels

_CACHE = {}


def _build_nc():
    f32 = mybir.dt.float32
    bf16 = mybir.dt.bfloat16

    nc = bacc.Bacc("TRN2", target_bir_lowering=False)

    xT = nc.declare_dram_parameter("xT", [DIM, L], bf16, isOutput=False)
    expmT = nc.declare_dram_parameter("expmT", [L, L], bf16, isOutput=False)
    wq = nc.declare_dram_parameter("wq", [DIM, GW], bf16, isOutput=False)
    wk = nc.declare_dram_parameter("wk", [DIM, GW], bf16, isOutput=False)
    wv = nc.declare_dram_parameter("wv", [DIM, GW], bf16, isOutput=False)
    outT = nc.declare_dram_parameter("outT", [GW, L], f32, isOutput=True)

    with tile.TileContext(nc) as tc:
        with (
            tc.tile_pool(name="persist", bufs=1) as persist,
            tc.tile_pool(name="em", bufs=34) as em_pool,
            tc.tile_pool(name="e", bufs=4) as e_pool,
            tc.tile_pool(name="eh", bufs=6) as eh_pool,
            tc.tile_pool(name="osb", bufs=2) as osb_pool,
            tc.tile_pool(name="res", bufs=2) as res_pool,
            # one PSUM pool, three tags, 8 banks total:
            #   s    2x [128,1024] (2 banks each) -> scores / phaseA kt+qt pairs
            #   o    2x [128, 512]                -> PV accum / phaseA V kb0-1
            #   proj 2x [128, 512]                -> deferred units / phaseA V kb2-3
            tc.tile_pool(name="ps", bufs=2, space="PSUM") as ps,
        ):
            # ---- persistent SBUF ----
            xt_sb = []
            w_sb = {"q": [], "k": [], "v": []}
            for kd in range(KD):
                t = persist.tile([P, L], bf16, tag=f"xt{kd}", name=f"xt{kd}")
                xt_sb.append(t)
                for name in ("k", "q", "v"):
                    w = persist.tile(
                        [P, GW], bf16, tag=f"w{name}{kd}", name=f"w{name}{kd}"
                    )
                    w_sb[name].append(w)

            qt_sb = [
                [
                    persist.tile([P, QP], bf16, tag=f"qt{p}_{j}", name=f"qt{p}_{j}")
                    for j in range(NQP)
                ]
                for p in range(2)
            ]
            kt_sb = [
                [
                    persist.tile([P, QP], bf16, tag=f"kt{p}_{j}", name=f"kt{p}_{j}")
                    for j in range(NQP)
                ]
                for p in range(2)
            ]

            # V_all[:, kb, h, 0:64] = V block; [..., 64:128] = 1.0 (ones for
            # the softmax-denominator rows of the PV matmul).
            v_all = persist.tile([P, NSEQ, HPC, P], bf16, tag="v_all")
            nc.vector.memset(v_all[:], 1.0)

            # ---- phase A: stream input DMA + kd-accumulated projections ----
            # W before xT per block so each kd's matmuls release as soon as
            # its (larger) xT slab lands.
            for kd in range(KD):
                for name, dram in (("k", wk), ("q", wq), ("v", wv)):
                    nc.sync.dma_start(w_sb[name][kd][:], dram[kd * P : (kd + 1) * P, :])
                nc.sync.dma_start(xt_sb[kd][:], xT[kd * P : (kd + 1) * P, :])

            def mm_qk(dest_ps, col0, wname, p, j, kd):
                nc.tensor.matmul(
                    dest_ps[:, col0 : col0 + QP],
                    lhsT=w_sb[wname][kd][:, p * P : (p + 1) * P],
                    rhs=xt_sb[kd][:, j * QP : (j + 1) * QP],
                    start=(kd == 0),
                    stop=(kd == KD - 1),
                )

            def mm_v(dest_ps, kb, kd):
                nc.tensor.matmul(
                    dest_ps[:, 0:GW],
                    lhsT=xt_sb[kd][:, kb * P : (kb + 1) * P],
                    rhs=w_sb["v"][kd][:],
                    start=(kd == 0),
                    stop=(kd == KD - 1),
                )

            def copy_v(pv, kb):
                nc.vector.tensor_copy(
                    out=v_all[:, kb, :, 0:HD],
                    in_=pv[:, 0:GW].rearrange("p (h d) -> p h d", h=HPC),
                )

            # Concurrently-accumulating PSUM (one open group per bank):
            ktp0 = ps.tile([P, 2 * QP], f32, tag="s", name="ktp0")   # kt kp0: p0|p1
            qtp0 = ps.tile([P, 2 * QP], f32, tag="s", name="qtp0")   # qt j0:  p0|p1
            vps = [
                ps.tile([P, QP], f32, tag="o", name="vp0"),          # V kb0
                ps.tile([P, QP], f32, tag="o", name="vp1"),          # V kb1
                ps.tile([P, QP], f32, tag="proj", name="vp2"),       # V kb2
                ps.tile([P, QP], f32, tag="proj", name="vp3"),       # V kb3
            ]
            for kd in range(KD):
                mm_qk(ktp0, 0, "k", 0, 0, kd)
                mm_qk(ktp0, QP, "k", 1, 0, kd)
                mm_qk(qtp0, 0, "q", 0, 0, kd)
                mm_qk(qtp0, QP, "q", 1, 0, kd)
                for kb in range(4):
                    mm_v(vps[kb], kb, kd)

            # Evacuate phase-A psum (DVE casts f32->bf16).
            nc.vector.tensor_copy(out=kt_sb[0][0][:], in_=ktp0[:, 0:QP])
            nc.vector.tensor_copy(out=kt_sb[1][0][:], in_=ktp0[:, QP : 2 * QP])
            nc.vector.tensor_copy(out=qt_sb[0][0][:], in_=qtp0[:, 0:QP])
            nc.vector.tensor_copy(out=qt_sb[1][0][:], in_=qtp0[:, QP : 2 * QP])
            for kb in range(4):
                copy_v(vps[kb], kb)

            # ---- deferred projection units (emitted into the attention
            # stream on an explicit schedule; each is one psum bank) ----
            def unit_kt(p, j):
                def emit():
                    pp = ps.tile([P, QP], f32, tag="proj", name="pp_kt")
                    for kd in range(KD):
                        mm_qk(pp, 0, "k", p, j, kd)
                    nc.vector.tensor_copy(out=kt_sb[p][j][:], in_=pp[:])
                return emit

            def unit_qt(p, j):
                def emit():
                    pp = ps.tile([P, QP], f32, tag="proj", name="pp_qt")
                    for kd in range(KD):
                        mm_qk(pp, 0, "q", p, j, kd)
                    nc.vector.tensor_copy(out=qt_sb[p][j][:], in_=pp[:])
                return emit

            def unit_v(kb):
                def emit():
                    pv = ps.tile([P, QP], f32, tag="proj", name="pp_v")
                    for kd in range(KD):
                        mm_v(pv, kb, kd)
                    copy_v(pv, kb)
                return emit

            # schedule[(hp, kb)] -> units to emit after that k-block's
            # matmuls, during attention panel j=0.  Consumers: kt kp at
            # hp0-kb=4kp, V kb at hp0-kb; every unit lands >=1 block early.
            sched0 = {
                (0, 0): [unit_kt(0, 1)],
                (0, 1): [unit_kt(1, 1)],
                (0, 2): [unit_v(4)],
                (0, 3): [unit_v(5)],
                (0, 4): [unit_v(6)],
                (0, 5): [unit_v(7)],
                (0, 6): [unit_kt(0, 2)],
                (0, 7): [unit_kt(1, 2), unit_v(8)],
                (0, 8): [unit_v(9)],
                (0, 9): [unit_v(10)],
                (0, 10): [unit_v(11)],
                (0, 11): [unit_kt(0, 3), unit_v(12)],
                (0, 12): [unit_kt(1, 3), unit_v(13)],
                (0, 13): [unit_v(14)],
                (0, 14): [unit_v(15)],
            }

            def em_dma(j):
                ems = []
                for kb in range(NSEQ):
                    t = em_pool.tile([P, QP], bf16, tag="em")
                    nc.sync.dma_start(
                        t[:], expmT[kb * P : (kb + 1) * P, j * QP : (j + 1) * QP]
                    )
                    ems.append(t)
                return ems

            em_tiles = {0: em_dma(0)}

            # ---- attention ----
            for j in range(NQP):
                em = em_tiles.pop(j)
                if j == 0:
                    sched = dict(sched0)
                elif j < NQP - 1:
                    # prefetch qt for panel j+1 early in this panel
                    sched = {
                        (0, 0): [unit_qt(0, j + 1)],
                        (0, 1): [unit_qt(1, j + 1)],
                    }
                else:
                    sched = {}
                for hp in range(2):  # head pair (2*hp, 2*hp+1)
                    if hp == 1 and j + 1 < NQP:
                        em_tiles[j + 1] = em_dma(j + 1)
                        if j == 0:
                            # qt j1 prefetched at the tail of panel 0
                            sched[(1, 0)] = [unit_qt(0, 1)]
                            sched[(1, 1)] = [unit_qt(1, 1)]
                    po = {
                        i: ps.tile([P, QP], f32, tag="o", name=f"po{i}")
                        for i in range(2)
                    }
                    for kb in range(NSEQ):
                        pss = ps.tile([P, 2 * QP], f32, tag="s", name="pss")
                        for i in range(2):
                            o = i * HD
                            kp, ko = divmod(kb, NSEQ // NQP)
                            nc.tensor.matmul(
                                pss[:, i * QP : (i + 1) * QP],
                                lhsT=kt_sb[hp][kp][o : o + HD, ko * P : (ko + 1) * P],
                                rhs=qt_sb[hp][j][o : o + HD, :],
                                start=True,
                                stop=True,
                                tile_position=(o, 0),
                            )
                        e = e_pool.tile([P, 2 * QP], bf16, tag="e")
                        nc.scalar.activation(
                            e[:], pss[:], mybir.ActivationFunctionType.Exp
                        )
                        for i in range(2):
                            h = 2 * hp + i
                            eh = eh_pool.tile([P, QP], bf16, tag="eh")
                            # NOTE: offloading some of these to GpSimd was
                            # tried and is a net loss — GpSimd TT is ~3x
                            # slower and its SBUF port traffic slows DVE's
                            # own multiplies from ~380ns to ~600ns.
                            nc.vector.tensor_tensor(
                                eh[:],
                                e[:, i * QP : (i + 1) * QP],
                                em[kb][:],
                                mybir.AluOpType.mult,
                            )
                            nc.tensor.matmul(
                                po[i][:],
                                lhsT=v_all[:, kb, h, :],
                                rhs=eh[:],
                                start=(kb == 0),
                                stop=(kb == NSEQ - 1),
                            )
                        for u in sched.get((hp, kb), ()):
                            u()
                    # Normalization, batched per head-pair:
                    # copy both heads' psum out first (frees the PSUM banks
                    # for the next head-pair's PV accumulation promptly),
                    # then one [64,1024] Ln/Exp reciprocal + one multiply.
                    osb = osb_pool.tile([P, 2 * QP], f32, tag="osb", name="osb")
                    for i in range(2):
                        nc.vector.tensor_copy(
                            osb[:, i * QP : (i + 1) * QP], po[i][:]
                        )
                    # operands of tensor_tensor must share a partition base,
                    # so shift the denominator rows down via an SBUF->SBUF
                    # DMA (address-based, shift is fine)
                    r_t = osb_pool.tile([HD, 2 * QP], f32, tag="r_t", name="r_t")
                    nc.sync.dma_start(r_t[:], osb[HD : 2 * HD, :])
                    # 1/r as exp(-ln(r)) on ScalarE: DVE's iterative
                    # reciprocal is ~8 cyc/elem (measured 3.3us/tile) and
                    # there is no hardware divide, while these two ACT
                    # passes land in ScalarE's panel-boundary idle gaps.
                    rc = osb_pool.tile([HD, 2 * QP], f32, tag="rc", name="rc")
                    nc.scalar.activation(
                        rc[:], r_t[:], mybir.ActivationFunctionType.Ln
                    )
                    nc.scalar.activation(
                        rc[:], rc[:], mybir.ActivationFunctionType.Exp,
                        scale=-1.0,
                    )
                    res = res_pool.tile([HD, 2 * QP], f32, tag="res")
                    nc.vector.tensor_tensor(
                        res[:], osb[0:HD, :], rc[:], mybir.AluOpType.mult
                    )
                    for i in range(2):
                        h = 2 * hp + i
                        nc.sync.dma_start(
                            outT[h * HD : (h + 1) * HD, j * QP : (j + 1) * QP],
                            res[:, i * QP : (i + 1) * QP],
                        )

    nc.compile()
    return nc


def _prep_in_maps(x, attention_mask, Wq, Wk, Wv):
    x = np.asarray(x, np.float32)
    attention_mask = np.asarray(attention_mask, np.float32)
    Wq = np.asarray(Wq, np.float32)
    Wk = np.asarray(Wk, np.float32)
    Wv = np.asarray(Wv, np.float32)

    xT_b = [np.ascontiguousarray(x[b].T).astype(BF16) for b in range(B)]
    expmT_b = [
        np.exp(MASK_SCALE * attention_mask[b].T, dtype=np.float32).astype(BF16)
        for b in range(B)
    ]
    in_maps = []
    for c in range(N_CORES):
        b, hg = divmod(c, HPC)
        sl = slice(hg * GW, (hg + 1) * GW)
        in_maps.append(
            {
                "xT": xT_b[b],
                "expmT": expmT_b[b],
                "wq": np.ascontiguousarray(Wq[:, sl] * SCALE).astype(BF16),
                "wk": np.ascontiguousarray(Wk[:, sl]).astype(BF16),
                "wv": np.ascontiguousarray(Wv[:, sl]).astype(BF16),
            }
        )
    return in_maps


def kernel(x, attention_mask, Wq, bq, Wk, bk, Wv, bv, **_unused):
    # bq/bk/bv are zeros per the problem spec and are not applied.
    if "nc" not in _CACHE:
        _CACHE["nc"] = _build_nc()
    nc = _CACHE["nc"]

    in_maps = _prep_in_maps(x, attention_mask, Wq, Wk, Wv)
    r = run_bass_kernel_spmd(nc, in_maps, core_ids=list(range(N_CORES)))
    _CACHE["last_results"] = r

    out = np.empty((B, L, DIM), np.float32)
    for c in range(N_CORES):
        b, hg = divmod(c, HPC)
        out[b, :, hg * GW : (hg + 1) * GW] = r.results[c]["outT"].T
    return out


# revision 14
# speedup vs baseline: 1.0106x; 1.0106x over previous
"""Multi-head attention (B=2, L=2048, DIM=1024, H=16) on 8 TRN2 NeuronCores.

Sharding: core c = (batch b = c//4, head-group hg = c%4 of 4 heads / 256 dims).
Data parallel over B, tensor parallel over heads; Q/K/V weights column-sharded.
Each core is fully independent (no collectives); host gathers the 8 output
shards.

Per-core layout trick: everything is computed transposed (seq on the free
axis) so no on-device transposes are needed:
  QT/KT [hd, seq]  <- matmul(lhsT=W_slice, rhs=xT)       (xT transposed on host)
  ST    [k, q]     <- matmul(lhsT=KT_head, rhs=QT_head)  (= scores transposed)
  E     = exp(ST)         (max-subtraction skipped: logits are N(0,1)-scaled,
                           mask only subtracts -> exp stays in [e^-65, e^5])
  Emask = E * exp(-60*mask)^T                            (mask exp'd on host)
  OT    [hd+ones, q] <- matmul(lhsT=[V | ones], rhs=Emask) accumulated over k;
                        rows 64..127 give the softmax denominator replicated,
                        so out = OT[0:64] / OT[64:128] needs no partition
                        broadcast.
The 1/sqrt(64) score scale is folded into Wq on the host.
Biases are zeros per the problem spec and are skipped.

Scheduling (v2): the PE is the bottleneck engine (~178us of matmul work),
so the kernel keeps it busy end to end:
  - Phase A streams the input DMA: per contraction block kd, the projection
    matmuls for KT(kp0), QT(j0) and V(kb0-3) are emitted right behind that
    block's loads, accumulating in 8 concurrently-live PSUM banks.  The PE
    starts ~2us after launch instead of waiting ~13us for the full 5.5MB
    of xT+W.  NOTE: PSUM accumulation groups are per-BANK — interleaving
    two open groups inside one bank corrupts results (measured), so every
    concurrently-accumulating group gets its own bank ([128,1024] tiles
    hold two groups only because their halves are bank-aligned).
  - The remaining projections (KT kp1-3, V kb4-15, QT j1-3) are "deferred
    units" emitted into the attention instruction stream on an explicit
    schedule (each unit lands before its first consumer), so panel
    boundaries never stall on a projection and the PE has no idle gaps.
  - The per-head normalization is batched per head-pair ([64,1024] Ln/Exp
    on ScalarE instead of 2x[64,512]) and the exp(-ln) trick keeps
    division off the DVE.
"""

import sys

for _p in ("/opt/trn_rl_repo",):
    if _p not in sys.path:
        sys.path.append(_p)

import numpy as np
import ml_dtypes

import concourse.tile as tile
from concourse import bacc, mybir
from concourse.bass_utils import run_bass_kernel_spmd


def _patch_act_tables():
    """Force every activation onto the one table set that holds both Exp
    and Ln, so the kernel pays a single ACT_TABLE_LOAD instead of
    thrashing between `exp_and_others` and `natural_log` at every
    normalization (measured 19 loads = ~24us).  Set ids must stay stable
    (they index act_info.json), so entries are kept and only their
    function sets are emptied.
    """
    import concourse.hw_specs as hw_specs

    orig = hw_specs.get_activation_tables

    def patched(arch):
        t = orig(arch)
        keep = "natural_log_exp_and_others"
        if keep not in t:
            return t
        return {k: (v if k == keep else set()) for k, v in t.items()}

    patched.__wrapped__ = orig
    bacc.get_activation_tables = patched


_patch_act_tables()

BF16 = ml_dtypes.bfloat16

B, L, DIM, H = 2, 2048, 1024, 16
HPC = 4          # heads per core
HD = DIM // H    # 64
GW = HPC * HD    # 256, head-group width per core
N_CORES = 8
MASK_SCALE = -60.0
SCALE = float(HD) ** -0.5

P = 128
KD = DIM // P        # 8  contraction blocks for projections
NSEQ = L // P        # 16 seq blocks (k blocks)
QP = 512             # q panel width
NQP = L // QP        # 4 q panels

_CACHE = {}


def _build_nc():
    f32 = mybir.dt.float32
    bf16 = mybir.dt.bfloat16

    nc = bacc.Bacc("TRN2", target_bir_lowering=False)

    xT = nc.declare_dram_parameter("xT", [DIM, L], bf16, isOutput=False)
    expmT = nc.declare_dram_parameter("expmT", [L, L], bf16, isOutput=False)
    wq = nc.declare_dram_parameter("wq", [DIM, GW], bf16, isOutput=False)
    wk = nc.declare_dram_parameter("wk", [DIM, GW], bf16, isOutput=False)
    wv = nc.declare_dram_parameter("wv", [DIM, GW], bf16, isOutput=False)
    outT = nc.declare_dram_parameter("outT", [GW, L], f32, isOutput=True)

    with tile.TileContext(nc) as tc:
        with (
            tc.tile_pool(name="persist", bufs=1) as persist,
            tc.tile_pool(name="em", bufs=9) as em_pool,
            tc.tile_pool(name="e", bufs=4) as e_pool,
            tc.tile_pool(name="eh", bufs=6) as eh_pool,
            tc.tile_pool(name="osb", bufs=2) as osb_pool,
            tc.tile_pool(name="res", bufs=2) as res_pool,
            # one PSUM pool, three tags, 8 banks total:
            #   s    2x [128,1024] (2 banks each) -> scores / phaseA kt+qt pairs
            #   o    2x [128, 512]                -> PV accum / phaseA V kb0-1
            #   proj 2x [128, 512]                -> deferred units / phaseA V kb2-3
            tc.tile_pool(name="ps", bufs=2, space="PSUM") as ps,
        ):
            # ---- persistent SBUF ----
            # xT as 4 kd-PAIR tiles and each W as one [128, KD, GW] tile:
            # every dma_start costs ~610ns of serialized DMA_DIRECT2D issue
            # on the SP queue (measured), so inputs are loaded with 7 big
            # instructions instead of 32 small ones.  Pair granularity keeps
            # the phase-A kd-streaming (in-queue FIFO -> pairs land in
            # order).
            xtp = [
                persist.tile([P, 2, L], bf16, tag=f"xtp{t}", name=f"xtp{t}")
                for t in range(KD // 2)
            ]
            w_sb = {
                n: persist.tile([P, KD, GW], bf16, tag=f"w{n}", name=f"w{n}")
                for n in ("k", "q", "v")
            }

            def xt_ap(kd):
                return xtp[kd // 2][:, kd % 2, :]

            qt_sb = [
                [
                    persist.tile([P, QP], bf16, tag=f"qt{p}_{j}", name=f"qt{p}_{j}")
                    for j in range(NQP)
                ]
                for p in range(2)
            ]
            kt_sb = [
                [
                    persist.tile([P, QP], bf16, tag=f"kt{p}_{j}", name=f"kt{p}_{j}")
                    for j in range(NQP)
                ]
                for p in range(2)
            ]

            # V_all[:, kb, h, 0:64] = V block; [..., 64:128] = 1.0 (ones for
            # the softmax-denominator rows of the PV matmul).
            v_all = persist.tile([P, NSEQ, HPC, P], bf16, tag="v_all")
            nc.vector.memset(v_all[:], 1.0)

            # ---- phase A: stream input DMA + kd-accumulated projections ----
            # W first (small, every projection needs it), then the xT pairs
            # in kd order.
            for name, dram in (("k", wk), ("q", wq), ("v", wv)):
                nc.sync.dma_start(
                    w_sb[name][:],
                    dram[:].rearrange("(kd p) g -> p kd g", p=P),
                )
            for t in range(KD // 2):
                nc.sync.dma_start(
                    xtp[t][:],
                    xT[2 * t * P : (2 * t + 2) * P, :].rearrange(
                        "(two p) s -> p two s", p=P
                    ),
                )

            def mm_qk(dest_ps, col0, wname, p, j, kd):
                nc.tensor.matmul(
                    dest_ps[:, col0 : col0 + QP],
                    lhsT=w_sb[wname][:, kd, p * P : (p + 1) * P],
                    rhs=xtp[kd // 2][:, kd % 2, j * QP : (j + 1) * QP],
                    start=(kd == 0),
                    stop=(kd == KD - 1),
                )

            def mm_v(dest_ps, kb, kd):
                nc.tensor.matmul(
                    dest_ps[:, 0:GW],
                    lhsT=xtp[kd // 2][:, kd % 2, kb * P : (kb + 1) * P],
                    rhs=w_sb["v"][:, kd, :],
                    start=(kd == 0),
                    stop=(kd == KD - 1),
                )

            def copy_v(pv, kb):
                nc.vector.tensor_copy(
                    out=v_all[:, kb, :, 0:HD],
                    in_=pv[:, 0:GW].rearrange("p (h d) -> p h d", h=HPC),
                )

            # Concurrently-accumulating PSUM (one open group per bank):
            ktp0 = ps.tile([P, 2 * QP], f32, tag="s", name="ktp0")   # kt kp0: p0|p1
            qtp0 = ps.tile([P, 2 * QP], f32, tag="s", name="qtp0")   # qt j0:  p0|p1
            vps = [
                ps.tile([P, QP], f32, tag="o", name="vp0"),          # V kb0
                ps.tile([P, QP], f32, tag="o", name="vp1"),          # V kb1
                ps.tile([P, QP], f32, tag="proj", name="vp2"),       # V kb2
                ps.tile([P, QP], f32, tag="proj", name="vp3"),       # V kb3
            ]
            for kd in range(KD):
                mm_qk(ktp0, 0, "k", 0, 0, kd)
                mm_qk(ktp0, QP, "k", 1, 0, kd)
                mm_qk(qtp0, 0, "q", 0, 0, kd)
                mm_qk(qtp0, QP, "q", 1, 0, kd)
                for kb in range(4):
                    mm_v(vps[kb], kb, kd)

            # Evacuate phase-A psum (DVE casts f32->bf16).
            nc.vector.tensor_copy(out=kt_sb[0][0][:], in_=ktp0[:, 0:QP])
            nc.vector.tensor_copy(out=kt_sb[1][0][:], in_=ktp0[:, QP : 2 * QP])
            nc.vector.tensor_copy(out=qt_sb[0][0][:], in_=qtp0[:, 0:QP])
            nc.vector.tensor_copy(out=qt_sb[1][0][:], in_=qtp0[:, QP : 2 * QP])
            for kb in range(4):
                copy_v(vps[kb], kb)

            # ---- deferred projection units (emitted into the attention
            # stream on an explicit schedule; each is one psum bank) ----
            def unit_kt(p, j):
                def emit():
                    pp = ps.tile([P, QP], f32, tag="proj", name="pp_kt")
                    for kd in range(KD):
                        mm_qk(pp, 0, "k", p, j, kd)
                    nc.vector.tensor_copy(out=kt_sb[p][j][:], in_=pp[:])
                return emit

            def unit_qt(p, j):
                def emit():
                    pp = ps.tile([P, QP], f32, tag="proj", name="pp_qt")
                    for kd in range(KD):
                        mm_qk(pp, 0, "q", p, j, kd)
                    nc.vector.tensor_copy(out=qt_sb[p][j][:], in_=pp[:])
                return emit

            def unit_v(kb):
                def emit():
                    pv = ps.tile([P, QP], f32, tag="proj", name="pp_v")
                    for kd in range(KD):
                        mm_v(pv, kb, kd)
                    copy_v(pv, kb)
                return emit

            # schedule[(hp, kb)] -> units to emit after that k-block's
            # matmuls, during attention panel j=0.  Consumers: kt kp at
            # hp0-kb=4kp, V kb at hp0-kb; every unit lands >=1 block early.
            sched0 = {
                (0, 0): [unit_kt(0, 1)],
                (0, 1): [unit_kt(1, 1)],
                (0, 2): [unit_v(4)],
                (0, 3): [unit_v(5)],
                (0, 4): [unit_v(6)],
                (0, 5): [unit_v(7)],
                (0, 6): [unit_kt(0, 2)],
                (0, 7): [unit_kt(1, 2), unit_v(8)],
                (0, 8): [unit_v(9)],
                (0, 9): [unit_v(10)],
                (0, 10): [unit_v(11)],
                (0, 11): [unit_kt(0, 3), unit_v(12)],
                (0, 12): [unit_kt(1, 3), unit_v(13)],
                (0, 13): [unit_v(14)],
                (0, 14): [unit_v(15)],
            }

            def em_dma(j):
                # 4 k-blocks per DMA instruction: 610ns of SP issue each,
                # so a panel costs 4 issues instead of 16.
                ems = []
                for c in range(NSEQ // 4):
                    t = em_pool.tile([P, 4, QP], bf16, tag="em")
                    nc.sync.dma_start(
                        t[:],
                        expmT[
                            4 * c * P : (4 * c + 4) * P, j * QP : (j + 1) * QP
                        ].rearrange("(kb p) s -> p kb s", p=P),
                    )
                    ems.append(t)
                return ems

            def em_ap(em, kb):
                return em[kb // 4][:, kb % 4, :]

            em_tiles = {0: em_dma(0)}

            # ---- attention ----
            for j in range(NQP):
                em = em_tiles.pop(j)
                if j == 0:
                    sched = dict(sched0)
                elif j < NQP - 1:
                    # prefetch qt for panel j+1 early in this panel
                    sched = {
                        (0, 0): [unit_qt(0, j + 1)],
                        (0, 1): [unit_qt(1, j + 1)],
                    }
                else:
                    sched = {}
                for hp in range(2):  # head pair (2*hp, 2*hp+1)
                    if hp == 1 and j + 1 < NQP:
                        em_tiles[j + 1] = em_dma(j + 1)
                        if j == 0:
                            # qt j1 prefetched at the tail of panel 0
                            sched[(1, 0)] = [unit_qt(0, 1)]
                            sched[(1, 1)] = [unit_qt(1, 1)]
                    po = {
                        i: ps.tile([P, QP], f32, tag="o", name=f"po{i}")
                        for i in range(2)
                    }
                    # Software-pipelined k-loop: emit scores(kb+1) on the PE
                    # queue BEFORE PV(kb), so the PE's in-order wait for
                    # eh(kb) (ScalarE exp + DVE mult latency) is covered by
                    # useful work instead of a ~1us bubble at every head-pair
                    # boundary and k-step.
                    def emit_scores(kb):
                        pss = ps.tile([P, 2 * QP], f32, tag="s", name="pss")
                        for i in range(2):
                            o = i * HD
                            kp, ko = divmod(kb, NSEQ // NQP)
                            nc.tensor.matmul(
                                pss[:, i * QP : (i + 1) * QP],
                                lhsT=kt_sb[hp][kp][o : o + HD, ko * P : (ko + 1) * P],
                                rhs=qt_sb[hp][j][o : o + HD, :],
                                start=True,
                                stop=True,
                                tile_position=(o, 0),
                            )
                        e = e_pool.tile([P, 2 * QP], bf16, tag="e")
                        nc.scalar.activation(
                            e[:], pss[:], mybir.ActivationFunctionType.Exp
                        )
                        return e

                    def emit_pv(e, kb):
                        for i in range(2):
                            h = 2 * hp + i
                            eh = eh_pool.tile([P, QP], bf16, tag="eh")
                            # NOTE: offloading some of these to GpSimd was
                            # tried and is a net loss — GpSimd TT is ~3x
                            # slower and its SBUF port traffic slows DVE's
                            # own multiplies from ~380ns to ~600ns.
                            nc.vector.tensor_tensor(
                                eh[:],
                                e[:, i * QP : (i + 1) * QP],
                                em_ap(em, kb),
                                mybir.AluOpType.mult,
                            )
                            nc.tensor.matmul(
                                po[i][:],
                                lhsT=v_all[:, kb, h, :],
                                rhs=eh[:],
                                start=(kb == 0),
                                stop=(kb == NSEQ - 1),
                            )

                    pend = None
                    for kb in range(NSEQ):
                        e = emit_scores(kb)
                        if pend is not None:
                            emit_pv(*pend)
                        pend = (e, kb)
                        for u in sched.get((hp, kb), ()):
                            u()
                    emit_pv(*pend)
                    # Normalization, batched per head-pair:
                    # copy both heads' psum out first (frees the PSUM banks
                    # for the next head-pair's PV accumulation promptly),
                    # then one [64,1024] Ln/Exp reciprocal + one multiply.
                    osb = osb_pool.tile([P, 2 * QP], f32, tag="osb", name="osb")
                    for i in range(2):
                        nc.vector.tensor_copy(
                            osb[:, i * QP : (i + 1) * QP], po[i][:]
                        )
                    # operands of tensor_tensor must share a partition base,
                    # so shift the denominator rows down via an SBUF->SBUF
                    # DMA (address-based, shift is fine)
                    r_t = osb_pool.tile([HD, 2 * QP], f32, tag="r_t", name="r_t")
                    nc.sync.dma_start(r_t[:], osb[HD : 2 * HD, :])
                    # 1/r as exp(-ln(r)) on ScalarE: DVE's iterative
                    # reciprocal is ~8 cyc/elem (measured 3.3us/tile) and
                    # there is no hardware divide, while these two ACT
                    # passes land in ScalarE's panel-boundary idle gaps.
                    rc = osb_pool.tile([HD, 2 * QP], f32, tag="rc", name="rc")
                    nc.scalar.activation(
                        rc[:], r_t[:], mybir.ActivationFunctionType.Ln
                    )
                    nc.scalar.activation(
                        rc[:], rc[:], mybir.ActivationFunctionType.Exp,
                        scale=-1.0,
                    )
                    res = res_pool.tile([HD, 2 * QP], f32, tag="res")
                    nc.vector.tensor_tensor(
                        res[:], osb[0:HD, :], rc[:], mybir.AluOpType.mult
                    )
                    # both heads' outputs in one DMA instruction
                    nc.sync.dma_start(
                        outT[
                            2 * hp * HD : (2 * hp + 2) * HD,
                            j * QP : (j + 1) * QP,
                        ].rearrange("(two d) q -> d two q", two=2),
                        res[:].rearrange("d (two q) -> d two q", two=2),
                    )

    nc.compile()
    return nc


def _prep_in_maps(x, attention_mask, Wq, Wk, Wv):
    x = np.asarray(x, np.float32)
    attention_mask = np.asarray(attention_mask, np.float32)
    Wq = np.asarray(Wq, np.float32)
    Wk = np.asarray(Wk, np.float32)
    Wv = np.asarray(Wv, np.float32)

    xT_b = [np.ascontiguousarray(x[b].T).astype(BF16) for b in range(B)]
    expmT_b = [
        np.exp(MASK_SCALE * attention_mask[b].T, dtype=np.float32).astype(BF16)
        for b in range(B)
    ]
    in_maps = []
    for c in range(N_CORES):
        b, hg = divmod(c, HPC)
        sl = slice(hg * GW, (hg + 1) * GW)
        in_maps.append(
            {
                "xT": xT_b[b],
                "expmT": expmT_b[b],
                "wq": np.ascontiguousarray(Wq[:, sl] * SCALE).astype(BF16),
                "wk": np.ascontiguousarray(Wk[:, sl]).astype(BF16),
                "wv": np.ascontiguousarray(Wv[:, sl]).astype(BF16),
            }
        )
    return in_maps


def kernel(x, attention_mask, Wq, bq, Wk, bk, Wv, bv, **_unused):
    # bq/bk/bv are zeros per the problem spec and are not applied.
    if "nc" not in _CACHE:
        _CACHE["nc"] = _build_nc()
    nc = _CACHE["nc"]

    in_maps = _prep_in_maps(x, attention_mask, Wq, Wk, Wv)
    r = run_bass_kernel_spmd(nc, in_maps, core_ids=list(range(N_CORES)))
    _CACHE["last_results"] = r

    out = np.empty((B, L, DIM), np.float32)
    for c in range(N_CORES):
        b, hg = divmod(c, HPC)
        out[b, :, hg * GW : (hg + 1) * GW] = r.results[c]["outT"].T
    return out
